# revision 11
# baseline (speedup 1.0000x reference)
"""Trainium2 Bass/Tile kernel for nn_DetourTransformer.

Data-parallel over the node dim N=16 across 8 NeuronCores (2 nodes/core).

Host-side preprocessing folds eval-mode BatchNorms into the linear weights and
LayerNorm gamma/beta into the surrounding matmuls, so the on-device layer is:

    qT = Wq^T hT + bq          (feature-major [D, S], fp32)
    kT = Wk^T hT + bk
    v  = h Wv + bv             (s-major [S, D], drained to bf16, +ones cols)
    per head: scoresT[t,s] = k_h^T q_h   (K=32 row-packed matmuls, fp32 PSUM)
              e = exp(scoresT)           (ScalarE, PSUM->SBUF bf16)
              am = e * maskT             (VectorE/GpSimd, bf16)
              valT += v_aug^T am         (col-packed) ; Z += 1^T am
    valT += v^T (1-m)T                   (mask correction: masked keys give exp(0)=1)
    val = transpose(valT); Z' = transpose(Z) + (S - mask_colsum)
    i1 = val/Z' + h ; z1 = standardize(i1)
    ff = z1 Wff~ + bff~                  (gamma/beta folded in)
    i2 = z1*gamma + ff ; z2 = standardize(i2)
    h' = z2*gamma + (h Wskip + bskip~)
Output: h[:, 0] after 4 layers -> [N, 256].
"""

import sys

sys.path.insert(0, "/opt/trn_rl_repo")

from contextlib import ExitStack

import numpy as np
import ml_dtypes

import concourse.bass as bass
import concourse.tile as tile
from concourse import bacc, mybir
from concourse.masks import make_identity
from concourse.bass_utils import run_bass_kernel_spmd

F32 = mybir.dt.float32
BF16 = mybir.dt.bfloat16
AF = mybir.ActivationFunctionType
OP = mybir.AluOpType

N, S, DIN, D, H, L, C = 16, 512, 128, 256, 8, 4, 32
NCORES = 8
NPC = N // NCORES  # nodes per core
EPS = 1e-5
SC = S // 128  # 4 s/t chunks
KC = D // 128  # 2 feature chunks

_cache = {}


# ---------------------------------------------------------------- host prep
def _fold_params(params):
    """Fold BN affine + LN gamma/beta into weights. Returns packed arrays."""

    def bnfold(W, b0, bn):
        s = bn["g"] / np.sqrt(bn["v"] + EPS)
        return W * s[None, :], (b0 - bn["m"]) * s + bn["b"]

    f32 = lambda a: np.ascontiguousarray(np.asarray(a, np.float32))

    def lhsT_pack(W):  # [256,256] -> [128, kc, dc, 128] (stationary form)
        return W.reshape(KC, 128, KC, 128).transpose(1, 0, 2, 3)

    def rhs_pack(W):  # [256,256] -> [128, kc, 256] (moving form)
        return W.reshape(KC, 128, D).transpose(1, 0, 2)

    def col_pack(b):  # [256] -> [128, dc]
        return b.reshape(KC, 128).T

    params = {
        "W_in": np.asarray(params["W_in"], np.float32),
        "b_in": np.asarray(params["b_in"], np.float32),
        "bn_in": {k: np.asarray(v, np.float32) for k, v in params["bn_in"].items()},
        "layers": [
            {
                k: (
                    np.asarray(v, np.float32)
                    if not isinstance(v, dict)
                    else {kk: np.asarray(vv, np.float32) for kk, vv in v.items()}
                )
                for k, v in p.items()
            }
            for p in params["layers"]
        ],
    }

    Win, bin_ = bnfold(params["W_in"], params["b_in"], params["bn_in"])
    out = {
        "win": f32(Win),  # [128, 256] (k=DIN=128 partitions)
        "bin": f32(col_pack(bin_)),
    }
    wq, wk, wv, wff, wsk = [], [], [], [], []
    bq, bk = [], []
    bvr, bffr, bskr, gr = [], [], [], []
    for p in params["layers"]:
        Wq, bq_ = bnfold(p["Wq"], p["bq"], p["bnq"])
        Wk, bk_ = bnfold(p["Wk"], p["bk"], p["bnk"])
        Wv, bv_ = bnfold(p["Wv"], p["bv"], p["bnv"])
        Wf, bf_ = bnfold(p["Wff"], p["bff"], p["bnff"])
        Ws, bs_ = bnfold(p["Wskip"], p["bskip"], p["bnskip"])
        g, b = p["ln_g"], p["ln_b"]
        Wf2 = g[:, None] * Wf
        bf2 = b @ Wf + bf_ + b
        bs2 = bs_ + b
        wq.append(lhsT_pack(Wq))
        wk.append(lhsT_pack(Wk))
        wv.append(rhs_pack(Wv))
        wff.append(rhs_pack(Wf2))
        wsk.append(rhs_pack(Ws))
        bq.append(col_pack(bq_))
        bk.append(col_pack(bk_))
        bvr.append(bv_)
        bffr.append(bf2)
        bskr.append(bs2)
        gr.append(g)
    out["wq"] = f32(np.stack(wq, 1))  # [128, L, kc, dc, 128]
    out["wk"] = f32(np.stack(wk, 1))
    out["wv"] = f32(np.stack(wv, 1))  # [128, L, kc, 256]
    out["wff"] = f32(np.stack(wff, 1))
    out["wsk"] = f32(np.stack(wsk, 1))
    out["bq"] = f32(np.stack(bq, 1))  # [128, L, dc]
    out["bk"] = f32(np.stack(bk, 1))
    out["brows"] = f32(np.stack([np.stack(bffr), np.stack(bskr)], 0)[None])  # [1, 2, L, 256]
    out["bv"] = f32(np.stack(bvr)[None])  # [1, L, 256] (broadcast-DMA'd)
    out["gamma"] = f32(np.stack(gr)[None])  # [1, L, 256]
    return out


def _prep_node_inputs(x, pad_mask):
    """Per-node input transforms. x [N,S,DIN] f32, pad_mask [N,S,S]."""
    bf = ml_dtypes.bfloat16
    x = np.asarray(x, np.float32)
    m = np.asarray(pad_mask, np.float32)
    # xT: [N, 128, 512]  (feature-major input)
    xT = np.ascontiguousarray(x.transpose(0, 2, 1))
    # maskT layout [N, 128, SC, 512]: [n, p, j, s] = mask[n, s, 128j+p]
    mT = m.transpose(0, 2, 1).reshape(N, SC, 128, S).transpose(0, 2, 1, 3)
    maskT = np.ascontiguousarray(mT.astype(bf))
    mask1mT = np.ascontiguousarray((1.0 - mT).astype(bf))
    # zcorr [N, 128, SC]: [n, p, j] = S - sum_t mask[n, s=128j+p, t]
    zc = (S - m.sum(2)).reshape(N, SC, 128).transpose(0, 2, 1)
    zcorr = np.ascontiguousarray(zc.astype(np.float32))
    return xT, maskT, mask1mT, zcorr


# ---------------------------------------------------------------- device build
def _build():
    nc = bacc.Bacc(
        "TRN2", target_bir_lowering=False, debug=False, enable_asserts=False
    )

    di = {}
    di["xT"] = nc.dram_tensor("xT", [NPC, 128, S], F32, kind="ExternalInput")
    di["maskT"] = nc.dram_tensor("maskT", [NPC, 128, SC, S], BF16, kind="ExternalInput")
    di["mask1mT"] = nc.dram_tensor(
        "mask1mT", [NPC, 128, SC, S], BF16, kind="ExternalInput"
    )
    di["zcorr"] = nc.dram_tensor("zcorr", [NPC, 128, SC], F32, kind="ExternalInput")
    di["win"] = nc.dram_tensor("win", [128, D], F32, kind="ExternalInput")
    di["bin"] = nc.dram_tensor("bin", [128, KC], F32, kind="ExternalInput")
    di["wq"] = nc.dram_tensor("wq", [128, L, KC, KC, 128], F32, kind="ExternalInput")
    di["wk"] = nc.dram_tensor("wk", [128, L, KC, KC, 128], F32, kind="ExternalInput")
    di["wv"] = nc.dram_tensor("wv", [128, L, KC, D], F32, kind="ExternalInput")
    di["wff"] = nc.dram_tensor("wff", [128, L, KC, D], F32, kind="ExternalInput")
    di["wsk"] = nc.dram_tensor("wsk", [128, L, KC, D], F32, kind="ExternalInput")
    di["bq"] = nc.dram_tensor("bq", [128, L, KC], F32, kind="ExternalInput")
    di["bk"] = nc.dram_tensor("bk", [128, L, KC], F32, kind="ExternalInput")
    di["brows"] = nc.dram_tensor("brows", [1, 2, L, D], F32, kind="ExternalInput")
    di["bv"] = nc.dram_tensor("bv", [1, L, D], F32, kind="ExternalInput")
    di["gamma"] = nc.dram_tensor("gamma", [1, L, D], F32, kind="ExternalInput")
    out_dram = nc.dram_tensor("out", [NPC, D], F32, kind="ExternalOutput")

    def bcast_ap(dram, rows=128):
        ap = dram.ap()
        return bass.AP(tensor=ap.tensor, offset=ap.offset, ap=[[0, rows]] + ap.ap[1:])

    with tile.TileContext(nc) as tc, ExitStack() as ctx:
        const = ctx.enter_context(tc.tile_pool(name="const", bufs=1))
        nodep = ctx.enter_context(tc.tile_pool(name="nodep", bufs=2))
        hp = ctx.enter_context(tc.tile_pool(name="hp", bufs=2))
        lp = ctx.enter_context(tc.tile_pool(name="lp", bufs=1))
        ap_ = ctx.enter_context(tc.tile_pool(name="attnp", bufs=2))
        pp = ctx.enter_context(tc.tile_pool(name="pp", bufs=4, space="PSUM"))

        # ---- constants / params (loaded once)
        sb = {}
        for name, shape in [
            ("win", [128, D]),
            ("bin", [128, KC]),
            ("wq", [128, L, KC, KC, 128]),
            ("wk", [128, L, KC, KC, 128]),
            ("wv", [128, L, KC, D]),
            ("wff", [128, L, KC, D]),
            ("wsk", [128, L, KC, D]),
            ("bq", [128, L, KC]),
            ("bk", [128, L, KC]),
        ]:
            sb[name] = const.tile(shape, F32, name=f"sb_{name}")
            nc.sync.dma_start(out=sb[name], in_=di[name].ap())
        sb_brows = const.tile([1, 2, L, D], F32)
        nc.sync.dma_start(out=sb_brows, in_=di["brows"].ap())
        sb_bvbc = const.tile([128, L, D], F32)
        nc.sync.dma_start(out=sb_bvbc, in_=bcast_ap(di["bv"]))
        sb_gbc = const.tile([128, L, D], F32)
        nc.sync.dma_start(out=sb_gbc, in_=bcast_ap(di["gamma"]))

        ident = const.tile([128, 128], F32)
        make_identity(nc, ident)
        ones32 = const.tile([128, 32], BF16)
        nc.vector.memset(ones32, 1.0)
        ones_row = const.tile([1, 128], F32)
        nc.vector.memset(ones_row, 1.0)
        eps_t = const.tile([128, 1], F32)
        nc.vector.memset(eps_t, EPS)

        for n in range(NPC):
            # ---- per-node inputs
            xT = nodep.tile([128, S], F32, tag="xT")
            nc.sync.dma_start(out=xT, in_=di["xT"].ap()[n])
            maskT = nodep.tile([128, SC, S], BF16, tag="maskT")
            nc.sync.dma_start(out=maskT, in_=di["maskT"].ap()[n])
            m1T = nodep.tile([128, SC, S], BF16, tag="m1T")
            nc.sync.dma_start(out=m1T, in_=di["mask1mT"].ap()[n])
            zcorr = nodep.tile([128, SC], F32, tag="zcorr")
            nc.sync.dma_start(out=zcorr, in_=di["zcorr"].ap()[n])

            # ---- input projection -> hT [128, kc, 512], h_nat [128, sc, 256]
            ps_h = pp.tile([128, 1024], F32, tag="big")
            for dc in range(KC):
                nc.tensor.matmul(
                    ps_h[:, 512 * dc : 512 * dc + 512],
                    sb["win"][:, 128 * dc : 128 * dc + 128],
                    xT,
                    start=True,
                    stop=True,
                )
            hT = hp.tile([128, KC, S], F32, tag="hT")
            for dc in range(KC):
                nc.vector.tensor_scalar_add(
                    hT[:, dc, :],
                    ps_h[:, 512 * dc : 512 * dc + 512],
                    sb["bin"][:, dc : dc + 1],
                )
            h_nat = hp.tile([128, SC, D], F32, tag="h_nat")
            ps_t = pp.tile([128, 1024], F32, tag="big")
            for sc in range(SC):
                for dc in range(KC):
                    nc.tensor.transpose(
                        ps_t[:, 256 * sc + 128 * dc : 256 * sc + 128 * dc + 128],
                        hT[:, dc, 128 * sc : 128 * sc + 128],
                        ident,
                    )
            for sc in range(SC):
                nc.vector.tensor_copy(
                    h_nat[:, sc, :], ps_t[:, 256 * sc : 256 * sc + 256]
                )

            for l in range(L):
                last = l == L - 1
                # ---- q/k projections (feature-major, fp32)
                ps_q = pp.tile([128, 1024], F32, tag="big")
                ps_k = pp.tile([128, 1024], F32, tag="big")
                for ps, w in ((ps_q, "wq"), (ps_k, "wk")):
                    for dc in range(KC):
                        for kc in range(KC):
                            nc.tensor.matmul(
                                ps[:, 512 * dc : 512 * dc + 512],
                                sb[w][:, l, kc, dc, :],
                                hT[:, kc, :],
                                start=(kc == 0),
                                stop=(kc == KC - 1),
                            )
                qT = lp.tile([128, KC, S], F32, tag="qT")
                kT = lp.tile([128, KC, S], F32, tag="kT")
                for t, ps, b in ((qT, ps_q, "bq"), (kT, ps_k, "bk")):
                    for dc in range(KC):
                        nc.vector.tensor_scalar_add(
                            t[:, dc, :],
                            ps[:, 512 * dc : 512 * dc + 512],
                            sb[b][:, l, dc : dc + 1],
                        )

                # ---- v projection (s-major) -> v_aug bf16 [128, j, h, 33]
                ps_v = pp.tile([128, 1024], F32, tag="big")
                for sc in range(SC):
                    for kc in range(KC):
                        nc.tensor.matmul(
                            ps_v[:, 256 * sc : 256 * sc + 256],
                            hT[:, kc, 128 * sc : 128 * sc + 128],
                            sb["wv"][:, l, kc, :],
                            start=(kc == 0),
                            stop=(kc == KC - 1),
                        )
                v_all = lp.tile([128, SC, D], BF16, tag="v_all")
                for sc in range(SC):
                    nc.vector.tensor_tensor(
                        v_all[:, sc, :],
                        ps_v[:, 256 * sc : 256 * sc + 256],
                        sb_bvbc[:, l, :],
                        OP.add,
                    )

                # ---- attention
                ps_valz = [
                    pp.tile([128, 1024], F32, tag="big", name=f"ps_valz{_g}")
                    for _g in range(2)
                ]
                # mask corrections: valT += v^T (1-m); Z gets zcorr later
                for hg in range(2):
                    for j in range(SC):
                        nc.tensor.matmul(
                            ps_valz[hg][:, 0:512],
                            v_all[:, j, 128 * hg : 128 * hg + 128],
                            m1T[:, j, :],
                            start=(j == 0),
                            stop=False,
                            skip_group_check=True,
                        )
                for h in range(H):
                    hg, hx = divmod(h, 4)
                    hp_ = 32 * hx
                    ps_sc = pp.tile([128, 1024], F32, tag="big")
                    ps_sc2 = pp.tile([128, 1024], F32, tag="big")
                    for j in range(SC):
                        nc.tensor.matmul(
                            (ps_sc, ps_sc2)[j // 2][
                                :, 512 * (j % 2) : 512 * (j % 2) + 512
                            ],
                            kT[hp_ : hp_ + 32, hg, 128 * j : 128 * j + 128],
                            qT[hp_ : hp_ + 32, hg, :],
                            start=True,
                            stop=True,
                            tile_position=(hp_, 0),
                        )
                    e = ap_.tile([128, SC, S], BF16, tag="e")
                    nc.scalar.activation(e[:, 0:2, :], ps_sc, AF.Exp)
                    nc.scalar.activation(e[:, 2:4, :], ps_sc2, AF.Exp)
                    am = ap_.tile([128, SC, S], BF16, tag="am")
                    eng = nc.vector if h % 2 else nc.gpsimd
                    eng.tensor_tensor(am, e, maskT, OP.mult)
                    for j in range(SC):
                        nc.tensor.matmul(
                            ps_valz[hg][hp_ : hp_ + 32, 0:512],
                            v_all[:, j, 32 * h : 32 * h + 32],
                            am[:, j, :],
                            start=False,
                            stop=(j == SC - 1),
                            tile_position=(0, hp_),
                            skip_group_check=True,
                        )
                        nc.tensor.matmul(
                            ps_valz[hg][hp_ : hp_ + 32, 512:1024],
                            ones32,
                            am[:, j, :],
                            start=(j == 0),
                            stop=(j == SC - 1),
                            tile_position=(0, hp_),
                            skip_group_check=True,
                        )

                # ---- transpose val + broadcast-Z to s-major
                fm = lp.tile([128, 2, 1024], F32, tag="fm")
                for hg in range(2):
                    nc.vector.tensor_copy(fm[:, hg, :], ps_valz[hg])
                ps_vt = pp.tile([128, 1024], F32, tag="big")
                ps_zt = pp.tile([128, 1024], F32, tag="big")
                for sc in range(SC):
                    for hg in range(2):
                        nc.tensor.transpose(
                            ps_vt[:, 256 * sc + 128 * hg : 256 * sc + 128 * hg + 128],
                            fm[:, hg, 128 * sc : 128 * sc + 128],
                            ident,
                        )
                        nc.tensor.transpose(
                            ps_zt[:, 256 * sc + 128 * hg : 256 * sc + 128 * hg + 128],
                            fm[:, hg, 512 + 128 * sc : 512 + 128 * sc + 128],
                            ident,
                        )
                rz = lp.tile([128, 1024], F32, tag="rz")
                for sc in range(SC):
                    nc.vector.tensor_scalar_add(
                        rz[:, 256 * sc : 256 * sc + 256],
                        ps_zt[:, 256 * sc : 256 * sc + 256],
                        zcorr[:, sc : sc + 1],
                    )
                nc.vector.reciprocal(rz, rz)

                # ---- i1 = val/Z + h ; z1 = standardize(i1)
                i1 = lp.tile([128, SC, D], F32, tag="i1")
                i1a = lp.tile([128, SC, D], F32, tag="i1a")
                for sc in range(SC):
                    nc.vector.tensor_tensor(
                        i1a[:, sc, :],
                        ps_vt[:, 256 * sc : 256 * sc + 256],
                        rz[:, 256 * sc : 256 * sc + 256],
                        OP.mult,
                    )
                    nc.gpsimd.tensor_add(i1[:, sc, :], i1a[:, sc, :], h_nat[:, sc, :])

                def layernorm(dst, src, tagp):
                    st = lp.tile([128, SC, 6], F32, tag=f"st{tagp}")
                    mv = lp.tile([128, SC, 2], F32, tag=f"mv{tagp}")
                    sd = lp.tile([128, SC], F32, tag=f"sd{tagp}")
                    for sc in range(SC):
                        nc.vector.bn_stats(st[:, sc, :], src[:, sc, :])
                        nc.vector.bn_aggr(mv[:, sc, :], st[:, sc, :])
                    nc.scalar.activation(
                        sd, mv[:, :, 1], AF.Sqrt, bias=eps_t
                    )
                    nc.vector.reciprocal(sd, sd)
                    for sc in range(SC):
                        nc.vector.tensor_scalar(
                            dst[:, sc, :],
                            src[:, sc, :],
                            mv[:, sc, 0:1],
                            sd[:, sc : sc + 1],
                            OP.subtract,
                            OP.mult,
                        )

                z1 = lp.tile([128, SC, D], F32, tag="z1")
                layernorm(z1, i1, "a")

                # ---- z1T (for ff matmul), ff, i2, z2
                ps_z1t = pp.tile([128, 1024], F32, tag="big")
                for sc in range(SC):
                    for kc in range(KC):
                        nc.tensor.transpose(
                            ps_z1t[:, 512 * kc + 128 * sc : 512 * kc + 128 * sc + 128],
                            z1[:, sc, 128 * kc : 128 * kc + 128],
                            ident,
                        )
                z1T = lp.tile([128, KC, S], F32, tag="z1T")
                for kc in range(KC):
                    nc.vector.tensor_copy(
                        z1T[:, kc, :], ps_z1t[:, 512 * kc : 512 * kc + 512]
                    )
                ps_ff = pp.tile([128, 1024], F32, tag="big")
                for sc in range(SC):
                    for kc in range(KC):
                        nc.tensor.matmul(
                            ps_ff[:, 256 * sc : 256 * sc + 256],
                            z1T[:, kc, 128 * sc : 128 * sc + 128],
                            sb["wff"][:, l, kc, :],
                            start=(kc == 0),
                            stop=False,
                        )
                    nc.tensor.matmul(
                        ps_ff[:, 256 * sc : 256 * sc + 256],
                        ones_row,
                        sb_brows[0:1, 0, l, :],
                        start=False,
                        stop=True,
                        skip_group_check=True,
                    )
                i2 = lp.tile([128, SC, D], F32, tag="i2")
                i2a = lp.tile([128, SC, D], F32, tag="i2a")
                for sc in range(SC):
                    nc.gpsimd.tensor_mul(i2a[:, sc, :], z1[:, sc, :], sb_gbc[:, l, :])
                    nc.vector.tensor_add(
                        i2[:, sc, :], i2a[:, sc, :], ps_ff[:, 256 * sc : 256 * sc + 256]
                    )
                z2 = lp.tile([128, SC, D], F32, tag="z2")
                layernorm(z2, i2, "b")

                # ---- skip path + next h
                ps_xr = pp.tile([128, 1024], F32, tag="big")
                for sc in range(SC):
                    for kc in range(KC):
                        nc.tensor.matmul(
                            ps_xr[:, 256 * sc : 256 * sc + 256],
                            hT[:, kc, 128 * sc : 128 * sc + 128],
                            sb["wsk"][:, l, kc, :],
                            start=(kc == 0),
                            stop=False,
                        )
                    nc.tensor.matmul(
                        ps_xr[:, 256 * sc : 256 * sc + 256],
                        ones_row,
                        sb_brows[0:1, 1, l, :],
                        start=False,
                        stop=True,
                        skip_group_check=True,
                    )
                h_next = hp.tile([128, SC, D], F32, tag="h_nat")
                hna = lp.tile([128, SC, D], F32, tag="hna")
                for sc in range(SC):
                    nc.gpsimd.tensor_mul(hna[:, sc, :], z2[:, sc, :], sb_gbc[:, l, :])
                    nc.vector.tensor_add(
                        h_next[:, sc, :], hna[:, sc, :],
                        ps_xr[:, 256 * sc : 256 * sc + 256],
                    )
                h_nat = h_next
                if not last:
                    hT_next = hp.tile([128, KC, S], F32, tag="hT")
                    ps_ht = pp.tile([128, 1024], F32, tag="big")
                    for sc in range(SC):
                        for kc in range(KC):
                            nc.tensor.transpose(
                                ps_ht[
                                    :, 512 * kc + 128 * sc : 512 * kc + 128 * sc + 128
                                ],
                                h_nat[:, sc, 128 * kc : 128 * kc + 128],
                                ident,
                            )
                    for kc in range(KC):
                        nc.vector.tensor_copy(
                            hT_next[:, kc, :], ps_ht[:, 512 * kc : 512 * kc + 512]
                        )
                    hT = hT_next

            nc.sync.dma_start(out=out_dram.ap()[n : n + 1, :], in_=h_nat[0:1, 0, :])

    nc.compile()
    return nc


# ---------------------------------------------------------------- entry point
def kernel(x, pad_mask, params):
    if "nc" not in _cache:
        _cache["nc"] = _build()
    nc = _cache["nc"]

    fp = _fold_params(params)
    xT, maskT, mask1mT, zcorr = _prep_node_inputs(x, pad_mask)

    in_maps = []
    for c in range(NCORES):
        sl = slice(c * NPC, (c + 1) * NPC)
        m = dict(fp_shared := {})
        m = {
            "xT": xT[sl],
            "maskT": maskT[sl],
            "mask1mT": mask1mT[sl],
            "zcorr": zcorr[sl],
            "win": fp["win"],
            "bin": fp["bin"],
            "wq": fp["wq"],
            "wk": fp["wk"],
            "wv": fp["wv"],
            "wff": fp["wff"],
            "wsk": fp["wsk"],
            "bq": fp["bq"],
            "bk": fp["bk"],
            "brows": fp["brows"],
            "bv": fp["bv"],
            "gamma": fp["gamma"],
        }
        in_maps.append(m)

    res = run_bass_kernel_spmd(nc, in_maps, list(range(NCORES)))
    outs = [np.asarray(res.results[c]["out"]) for c in range(NCORES)]
    return np.concatenate(outs, axis=0).astype(np.float32)
